# revision 21
# baseline (speedup 1.0000x reference)
"""Distributed Trainium2 kernel for 3D-RoPE GQA attention (nn_Attention_80530636800482).

Sharding: 8 cores = (batch b in {0,1}) x (kv group g in {0..3}).
Each core computes qkv projection for its 4 q-heads + 1 kv head, 3D RoPE,
attention over the full 2048-token sequence, and the partial output
projection for its 384 head-channels.  Host sums the 4 partial outputs
per batch (row-parallel w_o recombination) — pure data-parallel, no
collectives.

Device layout is dim-major [d, tokens] throughout:
  - qkvT [768pad, 2048] comes straight out of the projection matmuls
  - RoPE pair-swap is a 96x96 permutation matmul; cos/sin tables are
    host-precomputed [96, 2048] with the sign folded into sin
  - scores are computed transposed (keys on partitions) so the softmax
    denominator falls out of the attn@v matmul via a ones-row on V
  - no max-subtraction (scores are bounded ~|s|<15 for this data)
  - normalization by 1/den is deferred past attn@v and applied with a
    K=1 broadcast matmul + one elementwise multiply per tile
Matmuls run as float32r (full-rate fp32 on TRN2 for moving dim >=256,
~1.5e-4 rel err per K=128 contraction); every tensor feeding a matmul is
declared float32r so walrus sees rounded producers.
"""

import sys

if "/opt/trn_rl_repo" not in sys.path:
    sys.path.insert(0, "/opt/trn_rl_repo")

from contextlib import ExitStack

import numpy as np

import concourse.bass as bass
import concourse.tile as tile
from concourse import bacc, mybir
from concourse.bass_utils import run_bass_kernel_spmd

D_MODEL = 1536
NUM_HEADS = 16
QUERY_GROUPS = 4
HEAD_DIM = 96
HEADS_PER_GROUP = NUM_HEADS // QUERY_GROUPS  # 4
THETA = 10000.0
B = 2
N = 2048
NCH = 4          # 512-token chunks
TT = 16          # 128-token tiles
KD = 12          # 128-row contraction tiles of D_MODEL
SEC = 6          # q0 q1 q2 q3 k v sections, each 96 rows padded to 128
NH = HEADS_PER_GROUP
SCALE = 1.0 / float(np.sqrt(HEAD_DIM))

F32 = mybir.dt.float32
F32R = mybir.dt.float32r


def _build_graph():
    nc = bacc.Bacc(None, target_bir_lowering=False)
    act = mybir.ActivationFunctionType

    xT = nc.declare_dram_parameter("xT", [D_MODEL, N], F32R, isOutput=False)
    wqkvT = nc.declare_dram_parameter("wqkvT", [D_MODEL, SEC * 128], F32R, isOutput=False)
    wogT = nc.declare_dram_parameter("wogT", [NH * HEAD_DIM, D_MODEL], F32R, isOutput=False)
    cosT = nc.declare_dram_parameter("cosT", [HEAD_DIM, N], F32R, isOutput=False)
    sinT = nc.declare_dram_parameter("sinT", [HEAD_DIM, N], F32, isOutput=False)
    pswT = nc.declare_dram_parameter("pswT", [HEAD_DIM, HEAD_DIM], F32R, isOutput=False)
    ident = nc.declare_dram_parameter("ident", [128, 128], F32R, isOutput=False)
    out_ext = nc.declare_dram_parameter("out", [N, D_MODEL], F32, isOutput=True)

    with tile.TileContext(nc) as tc, ExitStack() as top:
        # tensors crossing phase A -> B
        cross_ab = top.enter_context(tc.tile_pool(name="cross_ab", bufs=1))
        rot = cross_ab.tile([HEAD_DIM, 5, N], F32R)          # rotated q0..q3, k
        # v tokens-major + ones col at 96, zero-pad to 128 for M-aligned matmul
        v_aug = cross_ab.tile([128, TT, 128], F32R)
        nc.vector.memset(v_aug[:, :, HEAD_DIM:128].bitcast(F32), 0.0)
        nc.vector.memset(v_aug[:, :, HEAD_DIM : HEAD_DIM + 1].bitcast(F32), 1.0)

        # ---------------- phase A: qkv projection + rope + v transpose ------
        with ExitStack() as sa:
            pa = sa.enter_context(tc.tile_pool(name="pa", bufs=1))
            xp = sa.enter_context(tc.tile_pool(name="xp", bufs=2))
            half = KD // 2

            # DMA issue order is what gates the first matmul: interleave the
            # first x chunk with the weight halves on the sync queue so the
            # k-loop can start after ~4MB instead of after all ~15MB of input.
            x0 = xp.tile([128, KD, 512], F32R, tag="x_nch")
            w_sb = pa.tile([128, KD, SEC * 128], F32R)
            for i in range(2):
                nc.sync.dma_start(
                    out=x0[:, i * half : (i + 1) * half, :],
                    in_=xT[i * half * 128 : (i + 1) * half * 128, 0:512].rearrange(
                        "(a p) n -> p a n", p=128
                    ),
                )
                nc.sync.dma_start(
                    out=w_sb[:, i * half : (i + 1) * half, :],
                    in_=wqkvT[i * half * 128 : (i + 1) * half * 128, :].rearrange(
                        "(a p) m -> p a m", p=128
                    ),
                )
            w_kts = [w_sb[:, kt, :] for kt in range(KD)]
            # small constants on the gpsimd queue, parallel with sync/scalar
            cos_sb = pa.tile([HEAD_DIM, N], F32R)
            nc.gpsimd.dma_start(out=cos_sb[:], in_=cosT[:])
            sin_sb = pa.tile([HEAD_DIM, N], F32)
            nc.gpsimd.dma_start(out=sin_sb[:], in_=sinT[:])
            psw_sb = pa.tile([HEAD_DIM, HEAD_DIM], F32R)
            nc.gpsimd.dma_start(out=psw_sb[:], in_=pswT[:])
            id_sb = pa.tile([128, 128], F32R)
            nc.gpsimd.dma_start(out=id_sb[:], in_=ident[:])
            secp = sa.enter_context(tc.tile_pool(name="secp", bufs=3))
            vsbp = sa.enter_context(tc.tile_pool(name="vsbp", bufs=2))
            tmpp = sa.enter_context(tc.tile_pool(name="tmpp", bufs=4))
            psq = sa.enter_context(tc.tile_pool(name="psq", bufs=3, space="PSUM"))
            pswp = sa.enter_context(tc.tile_pool(name="pswp", bufs=2, space="PSUM"))
            ptr = sa.enter_context(tc.tile_pool(name="ptr", bufs=2, space="PSUM"))

            for nch in range(NCH):
                ncsl = slice(nch * 512, (nch + 1) * 512)
                if nch == 0:
                    x_nch = x0
                else:
                    x_nch = xp.tile([128, KD, 512], F32R, tag="x_nch")
                    nc.scalar.dma_start(
                        out=x_nch[:],
                        in_=xT[:, ncsl].rearrange("(a p) n -> p a n", p=128),
                    )
                # section order k, v, q0..q3 so phase B's deps (k, v) are
                # ready as early as possible for cross-phase pipelining
                for s in range(SEC):
                    ps = psq.tile([128, 512], F32, tag="ps_qkv")
                    for kt in range(KD):
                        nc.tensor.matmul(
                            ps[:],
                            w_kts[kt][:, s * 128 : s * 128 + 128],
                            x_nch[:, kt, :],
                            start=(kt == 0),
                            stop=(kt == KD - 1),
                        )
                    if s != 1:
                        # q/k section: rot = sec*cos + (Psw@sec)*sin
                        rot_idx = 4 if s == 0 else s - 2
                        sec_sb = secp.tile([HEAD_DIM, 512], F32R, tag="sec")
                        nc.vector.tensor_copy(sec_sb[:], ps[0:HEAD_DIM, :])
                        sw = pswp.tile([HEAD_DIM, 512], F32, tag="sw")
                        nc.tensor.matmul(
                            sw[:], psw_sb[:], sec_sb[:], start=True, stop=True
                        )
                        t_a = tmpp.tile([HEAD_DIM, 512], F32, tag="ta")
                        nc.gpsimd.tensor_mul(t_a[:], sec_sb[:], cos_sb[:, ncsl])
                        t_b = tmpp.tile([HEAD_DIM, 512], F32, tag="tb")
                        nc.vector.tensor_mul(t_b[:], sw[:], sin_sb[:, ncsl])
                        nc.vector.tensor_add(rot[:, rot_idx, ncsl], t_a[:], t_b[:])
                    else:
                        # v section: transpose to tokens-major, append ones col
                        v_sb = vsbp.tile([HEAD_DIM, 512], F32R, tag="v_sb")
                        nc.scalar.copy(v_sb[:], ps[0:HEAD_DIM, :])
                        for c in range(4):
                            kt_tok = nch * 4 + c
                            pst = ptr.tile([128, HEAD_DIM], F32R, tag="pst")
                            nc.tensor.transpose(
                                pst[:],
                                v_sb[:, c * 128 : (c + 1) * 128],
                                id_sb[0:HEAD_DIM, 0:HEAD_DIM],
                            )
                            nc.scalar.copy(v_aug[:, kt_tok, 0:HEAD_DIM], pst[:])

        # ---------------- phases B+C: attention + output proj, per q-chunk --
        with ExitStack() as sbc:
            cross_bc = sbc.enter_context(tc.tile_pool(name="cross_bc", bufs=1))
            wog_sb = cross_bc.tile([HEAD_DIM, NH, D_MODEL], F32R)
            nc.scalar.dma_start(
                out=wog_sb[:], in_=wogT[:].rearrange("(h f) e -> f h e", f=HEAD_DIM)
            )

            attnp = sbc.enter_context(tc.tile_pool(name="attnp", bufs=2))
            probsp = sbc.enter_context(tc.tile_pool(name="probsp", bufs=4))
            arawp = sbc.enter_context(tc.tile_pool(name="arawp", bufs=2))
            recipp = sbc.enter_context(tc.tile_pool(name="recipp", bufs=2))
            bcp = sbc.enter_context(tc.tile_pool(name="bcp", bufs=2))
            outp = sbc.enter_context(tc.tile_pool(name="outp", bufs=3))
            pscore = sbc.enter_context(
                tc.tile_pool(name="pscore", bufs=2, space="PSUM")
            )
            pattn = sbc.enter_context(tc.tile_pool(name="pattn", bufs=2, space="PSUM"))
            po = sbc.enter_context(tc.tile_pool(name="po", bufs=2, space="PSUM"))

            for qc in range(NCH):
                qsl = slice(qc * 512, (qc + 1) * 512)
                attnq = attnp.tile([HEAD_DIM, NH, 512], F32R, tag="attnq")
                for h in range(NH):
                    a_ps = pattn.tile([128, 512], F32, tag="a_ps")
                    for k2 in range(TT // 2):
                        s_ps = pscore.tile([128, 1024], F32, tag="s_ps")
                        for j in range(2):
                            kt = 2 * k2 + j
                            nc.tensor.matmul(
                                s_ps[:, j * 512 : (j + 1) * 512],
                                rot[:, 4, kt * 128 : (kt + 1) * 128],
                                rot[:, h, qsl],
                                start=True,
                                stop=True,
                            )
                        probs = probsp.tile([128, 1024], F32R, tag="probs")
                        nc.scalar.activation(probs[:], s_ps[:], act.Exp, scale=SCALE)
                        for j in range(2):
                            kt = 2 * k2 + j
                            nc.tensor.matmul(
                                a_ps[:],
                                v_aug[:, kt, :],
                                probs[:, j * 512 : (j + 1) * 512],
                                start=(kt == 0),
                                stop=(kt == TT - 1),
                            )
                    # normalize: attnq[h] = raw * broadcast(1/den)
                    den_sb = recipp.tile([1, 512], F32, tag="den")
                    nc.vector.tensor_copy(
                        den_sb[:], a_ps[HEAD_DIM : HEAD_DIM + 1, :]
                    )
                    recip = recipp.tile([1, 512], F32, tag="recip")
                    nc.vector.reciprocal_approx_fast(recip[:], den_sb[:])
                    bc_sb = bcp.tile([HEAD_DIM, 512], F32, tag="bc")
                    nc.gpsimd.partition_broadcast(bc_sb[:], recip[:])
                    araw = arawp.tile([HEAD_DIM, 512], F32, tag="araw")
                    nc.vector.tensor_copy(araw[:], a_ps[0:HEAD_DIM, :])
                    nc.vector.tensor_mul(attnq[:, h, :], araw[:], bc_sb[:])

                # output projection for this q-chunk's 512 tokens
                for tl in range(4):
                    o_sb = outp.tile([128, D_MODEL], F32, tag="o_sb")
                    for e in range(3):
                        o_ps = po.tile([128, 512], F32, tag="o_ps")
                        for h in range(NH):
                            nc.tensor.matmul(
                                o_ps[:],
                                attnq[:, h, tl * 128 : (tl + 1) * 128],
                                wog_sb[:, h, e * 512 : (e + 1) * 512],
                                start=(h == 0),
                                stop=(h == NH - 1),
                            )
                        nc.vector.tensor_copy(o_sb[:, e * 512 : (e + 1) * 512], o_ps[:])
                    row0 = qc * 512 + tl * 128
                    nc.sync.dma_start(
                        out=out_ext[row0 : row0 + 128, :], in_=o_sb[:]
                    )

    nc.finalize()
    return nc


def _rope_tables(grid_t, grid_h, grid_w):
    """cos/sin tables [96, 2048], dim-major, sign folded into sin."""
    t, h, w = np.meshgrid(
        np.arange(grid_t), np.arange(grid_h), np.arange(grid_w), indexing="ij"
    )
    pos = np.stack([t.reshape(-1), h.reshape(-1), w.reshape(-1)], axis=-1).astype(
        np.float64
    )  # [N, 3]
    dpa = HEAD_DIM // 3  # 32
    npairs = dpa // 2  # 16
    freqs = 1.0 / (THETA ** (np.arange(npairs, dtype=np.float64) * 2.0 / dpa))
    cos = np.zeros((HEAD_DIM, pos.shape[0]), dtype=np.float64)
    sin = np.zeros((HEAD_DIM, pos.shape[0]), dtype=np.float64)
    for axis in range(3):
        ang = pos[:, axis][None, :] * freqs[:, None]  # [npairs, N]
        c, s = np.cos(ang), np.sin(ang)
        base = axis * dpa
        cos[base + 0 : base + dpa : 2] = c
        cos[base + 1 : base + dpa : 2] = c
        sin[base + 0 : base + dpa : 2] = -s
        sin[base + 1 : base + dpa : 2] = s
    return cos.astype(np.float32), sin.astype(np.float32)


def _pair_swap():
    p = np.zeros((HEAD_DIM, HEAD_DIM), dtype=np.float32)
    for i in range(HEAD_DIM // 2):
        p[2 * i, 2 * i + 1] = 1.0
        p[2 * i + 1, 2 * i] = 1.0
    return p


def _run(x, w_qkv, w_o, grid_t, grid_h, grid_w, trace=False):
    x = np.asarray(x, dtype=np.float32)
    w_qkv = np.asarray(w_qkv, dtype=np.float32)
    w_o = np.asarray(w_o, dtype=np.float32)

    cos, sin = _rope_tables(int(grid_t), int(grid_h), int(grid_w))
    psw = _pair_swap()
    ident = np.eye(128, dtype=np.float32)

    q_dim = NUM_HEADS * HEAD_DIM  # 1536
    kv_dim = QUERY_GROUPS * HEAD_DIM  # 384

    in_maps = []
    for core in range(8):
        b, g = core // 4, core % 4
        # sections q0..q3 (head g*4+j), k(group g), v(group g), padded to 128 rows
        secs = [
            w_qkv[q_dim + g * HEAD_DIM : q_dim + (g + 1) * HEAD_DIM],
            w_qkv[q_dim + kv_dim + g * HEAD_DIM : q_dim + kv_dim + (g + 1) * HEAD_DIM],
        ]
        for j in range(NH):
            h = g * NH + j
            secs.append(w_qkv[h * HEAD_DIM : (h + 1) * HEAD_DIM])
        wsec = np.zeros((SEC * 128, D_MODEL), dtype=np.float32)
        for s, rows in enumerate(secs):
            wsec[s * 128 : s * 128 + HEAD_DIM] = rows
        in_maps.append(
            {
                "xT": np.ascontiguousarray(x[b].T),
                "wqkvT": np.ascontiguousarray(wsec.T),
                "wogT": np.ascontiguousarray(
                    w_o[:, g * kv_dim : (g + 1) * kv_dim].T
                ),
                "cosT": cos,
                "sinT": sin,
                "pswT": psw,
                "ident": ident,
            }
        )

    nc = _build_graph()
    res = run_bass_kernel_spmd(nc, in_maps, core_ids=list(range(8)), trace=trace)

    out = np.zeros((B, N, D_MODEL), dtype=np.float32)
    for core in range(8):
        out[core // 4] += res.results[core]["out"]
    return out, res


def kernel(x, w_qkv, w_o, grid_t, grid_h, grid_w):
    return _run(x, w_qkv, w_o, grid_t, grid_h, grid_w)[0]


# revision 23
# speedup vs baseline: 1.0086x; 1.0086x over previous
"""Distributed Trainium2 kernel for 3D-RoPE GQA attention (nn_Attention_80530636800482).

Sharding: 8 cores = (batch b in {0,1}) x (kv group g in {0..3}).
Each core computes qkv projection for its 4 q-heads + 1 kv head, 3D RoPE,
attention over the full 2048-token sequence, and the partial output
projection for its 384 head-channels.  Host sums the 4 partial outputs
per batch (row-parallel w_o recombination) — pure data-parallel, no
collectives.

Device layout is dim-major [d, tokens] throughout:
  - qkvT [768pad, 2048] comes straight out of the projection matmuls
  - RoPE pair-swap is a 96x96 permutation matmul; cos/sin tables are
    host-precomputed [96, 2048] with the sign folded into sin
  - scores are computed transposed (keys on partitions) so the softmax
    denominator falls out of the attn@v matmul via a ones-row on V
  - no max-subtraction (scores are bounded ~|s|<15 for this data)
  - normalization by 1/den is deferred past attn@v and applied with a
    K=1 broadcast matmul + one elementwise multiply per tile
Matmuls run as float32r (full-rate fp32 on TRN2 for moving dim >=256,
~1.5e-4 rel err per K=128 contraction); every tensor feeding a matmul is
declared float32r so walrus sees rounded producers.
"""

import sys

if "/opt/trn_rl_repo" not in sys.path:
    sys.path.insert(0, "/opt/trn_rl_repo")

from contextlib import ExitStack

import numpy as np

import concourse.bass as bass
import concourse.tile as tile
from concourse import bacc, mybir
from concourse.bass_utils import run_bass_kernel_spmd

D_MODEL = 1536
NUM_HEADS = 16
QUERY_GROUPS = 4
HEAD_DIM = 96
HEADS_PER_GROUP = NUM_HEADS // QUERY_GROUPS  # 4
THETA = 10000.0
B = 2
N = 2048
NCH = 4          # 512-token chunks
TT = 16          # 128-token tiles
KD = 12          # 128-row contraction tiles of D_MODEL
SEC = 6          # q0 q1 q2 q3 k v sections, each 96 rows padded to 128
NH = HEADS_PER_GROUP
SCALE = 1.0 / float(np.sqrt(HEAD_DIM))

F32 = mybir.dt.float32
F32R = mybir.dt.float32r


def _build_graph():
    nc = bacc.Bacc(None, target_bir_lowering=False)
    act = mybir.ActivationFunctionType

    xT = nc.declare_dram_parameter("xT", [D_MODEL, N], F32R, isOutput=False)
    wqkvT = nc.declare_dram_parameter("wqkvT", [D_MODEL, SEC * 128], F32R, isOutput=False)
    wogT = nc.declare_dram_parameter("wogT", [NH * HEAD_DIM, D_MODEL], F32R, isOutput=False)
    cosT = nc.declare_dram_parameter("cosT", [HEAD_DIM, N], F32R, isOutput=False)
    sinT = nc.declare_dram_parameter("sinT", [HEAD_DIM, N], F32, isOutput=False)
    pswT = nc.declare_dram_parameter("pswT", [HEAD_DIM, HEAD_DIM], F32R, isOutput=False)
    ident = nc.declare_dram_parameter("ident", [128, 128], F32R, isOutput=False)
    out_ext = nc.declare_dram_parameter("out", [N, D_MODEL], F32, isOutput=True)

    with tile.TileContext(nc) as tc, ExitStack() as top:
        # tensors crossing phase A -> B
        cross_ab = top.enter_context(tc.tile_pool(name="cross_ab", bufs=1))
        rot = cross_ab.tile([HEAD_DIM, 5, N], F32R)          # rotated q0..q3, k
        # v tokens-major + ones col at 96, zero-pad to 128 for M-aligned matmul
        v_aug = cross_ab.tile([128, TT, 128], F32R)
        nc.vector.memset(v_aug[:, :, HEAD_DIM:128].bitcast(F32), 0.0)
        nc.vector.memset(v_aug[:, :, HEAD_DIM : HEAD_DIM + 1].bitcast(F32), 1.0)

        # ---------------- phase A: qkv projection + rope + v transpose ------
        with ExitStack() as sa:
            pa = sa.enter_context(tc.tile_pool(name="pa", bufs=1))
            xp = sa.enter_context(tc.tile_pool(name="xp", bufs=2))
            half = KD // 2

            # DMA issue order is what gates the first matmul: interleave the
            # first x chunk with the weight halves on the sync queue so the
            # k-loop can start after ~4MB instead of after all ~15MB of input.
            x0 = xp.tile([128, KD, 512], F32R, tag="x_nch")
            w_sb = pa.tile([128, KD, SEC * 128], F32R)
            for i in range(2):
                nc.sync.dma_start(
                    out=x0[:, i * half : (i + 1) * half, :],
                    in_=xT[i * half * 128 : (i + 1) * half * 128, 0:512].rearrange(
                        "(a p) n -> p a n", p=128
                    ),
                )
                nc.sync.dma_start(
                    out=w_sb[:, i * half : (i + 1) * half, :],
                    in_=wqkvT[i * half * 128 : (i + 1) * half * 128, :].rearrange(
                        "(a p) m -> p a m", p=128
                    ),
                )
            w_kts = [w_sb[:, kt, :] for kt in range(KD)]
            # small constants on the gpsimd queue, parallel with sync/scalar
            psw_sb = pa.tile([HEAD_DIM, HEAD_DIM], F32R)
            nc.gpsimd.dma_start(out=psw_sb[:], in_=pswT[:])
            id_sb = pa.tile([128, 128], F32R)
            nc.gpsimd.dma_start(out=id_sb[:], in_=ident[:])
            cos_sb = pa.tile([HEAD_DIM, N], F32R)
            nc.gpsimd.dma_start(out=cos_sb[:], in_=cosT[:])
            sin_sb = pa.tile([HEAD_DIM, N], F32)
            nc.gpsimd.dma_start(out=sin_sb[:], in_=sinT[:])
            secp = sa.enter_context(tc.tile_pool(name="secp", bufs=3))
            vsbp = sa.enter_context(tc.tile_pool(name="vsbp", bufs=2))
            tmpp = sa.enter_context(tc.tile_pool(name="tmpp", bufs=4))
            psq = sa.enter_context(tc.tile_pool(name="psq", bufs=3, space="PSUM"))
            pswp = sa.enter_context(tc.tile_pool(name="pswp", bufs=2, space="PSUM"))
            ptr = sa.enter_context(tc.tile_pool(name="ptr", bufs=2, space="PSUM"))

            x_tiles = [x0, None, None, None]
            for nch in range(NCH):
                ncsl = slice(nch * 512, (nch + 1) * 512)
                x_nch = x_tiles[nch]
                # section order k, v, q0..q3 so phase B's deps (k, v) are
                # ready as early as possible for cross-phase pipelining
                for s in range(SEC):
                    if s == 2 and nch + 1 < NCH:
                        # prefetch next x chunk; positioned here in the scalar
                        # queue so it issues after this nch's v copies run —
                        # keeps early DMA bandwidth for the startup-critical
                        # w/x0 transfers
                        nxt = xp.tile([128, KD, 512], F32R, tag="x_nch")
                        nc.scalar.dma_start(
                            out=nxt[:],
                            in_=xT[:, (nch + 1) * 512 : (nch + 2) * 512].rearrange(
                                "(a p) n -> p a n", p=128
                            ),
                        )
                        x_tiles[nch + 1] = nxt
                    ps = psq.tile([128, 512], F32, tag="ps_qkv")
                    for kt in range(KD):
                        nc.tensor.matmul(
                            ps[:],
                            w_kts[kt][:, s * 128 : s * 128 + 128],
                            x_nch[:, kt, :],
                            start=(kt == 0),
                            stop=(kt == KD - 1),
                        )
                    if s != 1:
                        # q/k section: rot = sec*cos + (Psw@sec)*sin
                        rot_idx = 4 if s == 0 else s - 2
                        sec_sb = secp.tile([HEAD_DIM, 512], F32R, tag="sec")
                        nc.vector.tensor_copy(sec_sb[:], ps[0:HEAD_DIM, :])
                        sw = pswp.tile([HEAD_DIM, 512], F32, tag="sw")
                        nc.tensor.matmul(
                            sw[:], psw_sb[:], sec_sb[:], start=True, stop=True
                        )
                        t_a = tmpp.tile([HEAD_DIM, 512], F32, tag="ta")
                        nc.gpsimd.tensor_mul(t_a[:], sec_sb[:], cos_sb[:, ncsl])
                        t_b = tmpp.tile([HEAD_DIM, 512], F32, tag="tb")
                        nc.vector.tensor_mul(t_b[:], sw[:], sin_sb[:, ncsl])
                        nc.vector.tensor_add(rot[:, rot_idx, ncsl], t_a[:], t_b[:])
                    else:
                        # v section: transpose to tokens-major, append ones col
                        v_sb = vsbp.tile([HEAD_DIM, 512], F32R, tag="v_sb")
                        nc.scalar.copy(v_sb[:], ps[0:HEAD_DIM, :])
                        for c in range(4):
                            kt_tok = nch * 4 + c
                            pst = ptr.tile([128, HEAD_DIM], F32R, tag="pst")
                            nc.tensor.transpose(
                                pst[:],
                                v_sb[:, c * 128 : (c + 1) * 128],
                                id_sb[0:HEAD_DIM, 0:HEAD_DIM],
                            )
                            nc.scalar.copy(v_aug[:, kt_tok, 0:HEAD_DIM], pst[:])

        # ---------------- phases B+C: attention + output proj, per q-chunk --
        with ExitStack() as sbc:
            cross_bc = sbc.enter_context(tc.tile_pool(name="cross_bc", bufs=1))
            wog_sb = cross_bc.tile([HEAD_DIM, NH, D_MODEL], F32R)
            nc.scalar.dma_start(
                out=wog_sb[:], in_=wogT[:].rearrange("(h f) e -> f h e", f=HEAD_DIM)
            )

            attnp = sbc.enter_context(tc.tile_pool(name="attnp", bufs=2))
            probsp = sbc.enter_context(tc.tile_pool(name="probsp", bufs=4))
            arawp = sbc.enter_context(tc.tile_pool(name="arawp", bufs=2))
            recipp = sbc.enter_context(tc.tile_pool(name="recipp", bufs=2))
            bcp = sbc.enter_context(tc.tile_pool(name="bcp", bufs=2))
            outp = sbc.enter_context(tc.tile_pool(name="outp", bufs=3))
            pscore = sbc.enter_context(
                tc.tile_pool(name="pscore", bufs=2, space="PSUM")
            )
            pattn = sbc.enter_context(tc.tile_pool(name="pattn", bufs=2, space="PSUM"))
            po = sbc.enter_context(tc.tile_pool(name="po", bufs=2, space="PSUM"))

            for qc in range(NCH):
                qsl = slice(qc * 512, (qc + 1) * 512)
                attnq = attnp.tile([HEAD_DIM, NH, 512], F32R, tag="attnq")
                for h in range(NH):
                    a_ps = pattn.tile([128, 512], F32, tag="a_ps")
                    for k2 in range(TT // 2):
                        s_ps = pscore.tile([128, 1024], F32, tag="s_ps")
                        for j in range(2):
                            kt = 2 * k2 + j
                            nc.tensor.matmul(
                                s_ps[:, j * 512 : (j + 1) * 512],
                                rot[:, 4, kt * 128 : (kt + 1) * 128],
                                rot[:, h, qsl],
                                start=True,
                                stop=True,
                            )
                        probs = probsp.tile([128, 1024], F32R, tag="probs")
                        nc.scalar.activation(probs[:], s_ps[:], act.Exp, scale=SCALE)
                        for j in range(2):
                            kt = 2 * k2 + j
                            nc.tensor.matmul(
                                a_ps[:],
                                v_aug[:, kt, :],
                                probs[:, j * 512 : (j + 1) * 512],
                                start=(kt == 0),
                                stop=(kt == TT - 1),
                            )
                    # normalize: attnq[h] = raw * broadcast(1/den)
                    den_sb = recipp.tile([1, 512], F32, tag="den")
                    nc.vector.tensor_copy(
                        den_sb[:], a_ps[HEAD_DIM : HEAD_DIM + 1, :]
                    )
                    recip = recipp.tile([1, 512], F32, tag="recip")
                    nc.vector.reciprocal_approx_fast(recip[:], den_sb[:])
                    bc_sb = bcp.tile([HEAD_DIM, 512], F32, tag="bc")
                    nc.gpsimd.partition_broadcast(bc_sb[:], recip[:])
                    araw = arawp.tile([HEAD_DIM, 512], F32, tag="araw")
                    nc.vector.tensor_copy(araw[:], a_ps[0:HEAD_DIM, :])
                    nc.vector.tensor_mul(attnq[:, h, :], araw[:], bc_sb[:])

                # output projection for this q-chunk's 512 tokens
                for tl in range(4):
                    o_sb = outp.tile([128, D_MODEL], F32, tag="o_sb")
                    for e in range(3):
                        o_ps = po.tile([128, 512], F32, tag="o_ps")
                        for h in range(NH):
                            nc.tensor.matmul(
                                o_ps[:],
                                attnq[:, h, tl * 128 : (tl + 1) * 128],
                                wog_sb[:, h, e * 512 : (e + 1) * 512],
                                start=(h == 0),
                                stop=(h == NH - 1),
                            )
                        nc.vector.tensor_copy(o_sb[:, e * 512 : (e + 1) * 512], o_ps[:])
                    row0 = qc * 512 + tl * 128
                    nc.sync.dma_start(
                        out=out_ext[row0 : row0 + 128, :], in_=o_sb[:]
                    )

    nc.finalize()
    return nc


def _rope_tables(grid_t, grid_h, grid_w):
    """cos/sin tables [96, 2048], dim-major, sign folded into sin."""
    t, h, w = np.meshgrid(
        np.arange(grid_t), np.arange(grid_h), np.arange(grid_w), indexing="ij"
    )
    pos = np.stack([t.reshape(-1), h.reshape(-1), w.reshape(-1)], axis=-1).astype(
        np.float64
    )  # [N, 3]
    dpa = HEAD_DIM // 3  # 32
    npairs = dpa // 2  # 16
    freqs = 1.0 / (THETA ** (np.arange(npairs, dtype=np.float64) * 2.0 / dpa))
    cos = np.zeros((HEAD_DIM, pos.shape[0]), dtype=np.float64)
    sin = np.zeros((HEAD_DIM, pos.shape[0]), dtype=np.float64)
    for axis in range(3):
        ang = pos[:, axis][None, :] * freqs[:, None]  # [npairs, N]
        c, s = np.cos(ang), np.sin(ang)
        base = axis * dpa
        cos[base + 0 : base + dpa : 2] = c
        cos[base + 1 : base + dpa : 2] = c
        sin[base + 0 : base + dpa : 2] = -s
        sin[base + 1 : base + dpa : 2] = s
    return cos.astype(np.float32), sin.astype(np.float32)


def _pair_swap():
    p = np.zeros((HEAD_DIM, HEAD_DIM), dtype=np.float32)
    for i in range(HEAD_DIM // 2):
        p[2 * i, 2 * i + 1] = 1.0
        p[2 * i + 1, 2 * i] = 1.0
    return p


def _run(x, w_qkv, w_o, grid_t, grid_h, grid_w, trace=False):
    x = np.asarray(x, dtype=np.float32)
    w_qkv = np.asarray(w_qkv, dtype=np.float32)
    w_o = np.asarray(w_o, dtype=np.float32)

    cos, sin = _rope_tables(int(grid_t), int(grid_h), int(grid_w))
    psw = _pair_swap()
    ident = np.eye(128, dtype=np.float32)

    q_dim = NUM_HEADS * HEAD_DIM  # 1536
    kv_dim = QUERY_GROUPS * HEAD_DIM  # 384

    in_maps = []
    for core in range(8):
        b, g = core // 4, core % 4
        # sections q0..q3 (head g*4+j), k(group g), v(group g), padded to 128 rows
        secs = [
            w_qkv[q_dim + g * HEAD_DIM : q_dim + (g + 1) * HEAD_DIM],
            w_qkv[q_dim + kv_dim + g * HEAD_DIM : q_dim + kv_dim + (g + 1) * HEAD_DIM],
        ]
        for j in range(NH):
            h = g * NH + j
            secs.append(w_qkv[h * HEAD_DIM : (h + 1) * HEAD_DIM])
        wsec = np.zeros((SEC * 128, D_MODEL), dtype=np.float32)
        for s, rows in enumerate(secs):
            wsec[s * 128 : s * 128 + HEAD_DIM] = rows
        in_maps.append(
            {
                "xT": np.ascontiguousarray(x[b].T),
                "wqkvT": np.ascontiguousarray(wsec.T),
                "wogT": np.ascontiguousarray(
                    w_o[:, g * kv_dim : (g + 1) * kv_dim].T
                ),
                "cosT": cos,
                "sinT": sin,
                "pswT": psw,
                "ident": ident,
            }
        )

    nc = _build_graph()
    res = run_bass_kernel_spmd(nc, in_maps, core_ids=list(range(8)), trace=trace)

    out = np.zeros((B, N, D_MODEL), dtype=np.float32)
    for core in range(8):
        out[core // 4] += res.results[core]["out"]
    return out, res


def kernel(x, w_qkv, w_o, grid_t, grid_h, grid_w):
    return _run(x, w_qkv, w_o, grid_t, grid_h, grid_w)[0]


# revision 24
# speedup vs baseline: 1.1380x; 1.1284x over previous
"""Distributed Trainium2 kernel for 3D-RoPE GQA attention (nn_Attention_80530636800482).

Sharding: 8 cores = (batch b in {0,1}) x (kv group g in {0..3}).
Each core computes qkv projection for its 4 q-heads + 1 kv head, 3D RoPE,
attention over the full 2048-token sequence, and the partial output
projection for its 384 head-channels.  Host sums the 4 partial outputs
per batch (row-parallel w_o recombination) — pure data-parallel, no
collectives.

Device layout is dim-major [d, tokens] throughout:
  - qkvT [768pad, 2048] comes straight out of the projection matmuls
  - RoPE pair-swap is a 96x96 permutation matmul; cos/sin tables are
    host-precomputed [96, 2048] with the sign folded into sin
  - scores are computed transposed (keys on partitions) so the softmax
    denominator falls out of the attn@v matmul via a ones-row on V
  - no max-subtraction (scores are bounded ~|s|<15 for this data)
  - normalization by 1/den is deferred past attn@v and applied with a
    K=1 broadcast matmul + one elementwise multiply per tile
Matmuls run as float32r (full-rate fp32 on TRN2 for moving dim >=256,
~1.5e-4 rel err per K=128 contraction); every tensor feeding a matmul is
declared float32r so walrus sees rounded producers.
"""

import sys

if "/opt/trn_rl_repo" not in sys.path:
    sys.path.insert(0, "/opt/trn_rl_repo")

from contextlib import ExitStack

import numpy as np

import concourse.bass as bass
import concourse.tile as tile
from concourse import bacc, mybir
from concourse.bass_utils import run_bass_kernel_spmd

D_MODEL = 1536
NUM_HEADS = 16
QUERY_GROUPS = 4
HEAD_DIM = 96
HEADS_PER_GROUP = NUM_HEADS // QUERY_GROUPS  # 4
THETA = 10000.0
B = 2
N = 2048
NCH = 4          # 512-token chunks
TT = 16          # 128-token tiles
KD = 12          # 128-row contraction tiles of D_MODEL
SEC = 6          # q0 q1 q2 q3 k v sections, each 96 rows padded to 128
NH = HEADS_PER_GROUP
SCALE = 1.0 / float(np.sqrt(HEAD_DIM))

F32 = mybir.dt.float32
F32R = mybir.dt.bfloat16  # matmul operand dtype (bf16: half DMA, FWL)


def _build_graph():
    nc = bacc.Bacc(None, target_bir_lowering=False)
    act = mybir.ActivationFunctionType

    xT = nc.declare_dram_parameter("xT", [D_MODEL, N], F32R, isOutput=False)
    wqkvT = nc.declare_dram_parameter("wqkvT", [D_MODEL, SEC * 128], F32R, isOutput=False)
    wogT = nc.declare_dram_parameter("wogT", [NH * HEAD_DIM, D_MODEL], F32R, isOutput=False)
    cosT = nc.declare_dram_parameter("cosT", [HEAD_DIM, N], F32R, isOutput=False)
    sinT = nc.declare_dram_parameter("sinT", [HEAD_DIM, N], F32, isOutput=False)
    pswT = nc.declare_dram_parameter("pswT", [HEAD_DIM, HEAD_DIM], F32R, isOutput=False)
    ident = nc.declare_dram_parameter("ident", [128, 128], F32R, isOutput=False)
    out_ext = nc.declare_dram_parameter("out", [N, D_MODEL], F32, isOutput=True)

    with tile.TileContext(nc) as tc, ExitStack() as top:
        # tensors crossing phase A -> B
        cross_ab = top.enter_context(tc.tile_pool(name="cross_ab", bufs=1))
        rot = cross_ab.tile([HEAD_DIM, 5, N], F32R)          # rotated q0..q3, k
        # v tokens-major + ones col at 96, zero-pad to 128 for M-aligned matmul
        v_aug = cross_ab.tile([128, TT, 128], F32R)
        nc.vector.memset(v_aug[:, :, HEAD_DIM:128], 0.0)
        nc.vector.memset(v_aug[:, :, HEAD_DIM : HEAD_DIM + 1], 1.0)

        # ---------------- phase A: qkv projection + rope + v transpose ------
        with ExitStack() as sa:
            pa = sa.enter_context(tc.tile_pool(name="pa", bufs=1))
            xp = sa.enter_context(tc.tile_pool(name="xp", bufs=2))
            half = KD // 2

            # DMA issue order is what gates the first matmul: interleave the
            # first x chunk with the weight halves on the sync queue so the
            # k-loop can start after ~4MB instead of after all ~15MB of input.
            x0 = xp.tile([128, KD, 512], F32R, tag="x_nch")
            w_sb = pa.tile([128, KD, SEC * 128], F32R)
            for i in range(2):
                nc.sync.dma_start(
                    out=x0[:, i * half : (i + 1) * half, :],
                    in_=xT[i * half * 128 : (i + 1) * half * 128, 0:512].rearrange(
                        "(a p) n -> p a n", p=128
                    ),
                )
                nc.sync.dma_start(
                    out=w_sb[:, i * half : (i + 1) * half, :],
                    in_=wqkvT[i * half * 128 : (i + 1) * half * 128, :].rearrange(
                        "(a p) m -> p a m", p=128
                    ),
                )
            w_kts = [w_sb[:, kt, :] for kt in range(KD)]
            # small constants on the gpsimd queue, parallel with sync/scalar
            psw_sb = pa.tile([HEAD_DIM, HEAD_DIM], F32R)
            nc.gpsimd.dma_start(out=psw_sb[:], in_=pswT[:])
            id_sb = pa.tile([128, 128], F32R)
            nc.gpsimd.dma_start(out=id_sb[:], in_=ident[:])
            cos_sb = pa.tile([HEAD_DIM, N], F32R)
            nc.gpsimd.dma_start(out=cos_sb[:], in_=cosT[:])
            sin_sb = pa.tile([HEAD_DIM, N], F32)
            nc.gpsimd.dma_start(out=sin_sb[:], in_=sinT[:])
            secp = sa.enter_context(tc.tile_pool(name="secp", bufs=3))
            vsbp = sa.enter_context(tc.tile_pool(name="vsbp", bufs=2))
            tmpp = sa.enter_context(tc.tile_pool(name="tmpp", bufs=4))
            psq = sa.enter_context(tc.tile_pool(name="psq", bufs=3, space="PSUM"))
            pswp = sa.enter_context(tc.tile_pool(name="pswp", bufs=2, space="PSUM"))
            ptr = sa.enter_context(tc.tile_pool(name="ptr", bufs=2, space="PSUM"))

            x_tiles = [x0, None, None, None]
            for nch in range(NCH):
                ncsl = slice(nch * 512, (nch + 1) * 512)
                x_nch = x_tiles[nch]
                # section order k, v, q0..q3 so phase B's deps (k, v) are
                # ready as early as possible for cross-phase pipelining
                for s in range(SEC):
                    if s == 2 and nch + 1 < NCH:
                        # prefetch next x chunk; positioned here in the scalar
                        # queue so it issues after this nch's v copies run —
                        # keeps early DMA bandwidth for the startup-critical
                        # w/x0 transfers
                        nxt = xp.tile([128, KD, 512], F32R, tag="x_nch")
                        nc.scalar.dma_start(
                            out=nxt[:],
                            in_=xT[:, (nch + 1) * 512 : (nch + 2) * 512].rearrange(
                                "(a p) n -> p a n", p=128
                            ),
                        )
                        x_tiles[nch + 1] = nxt
                    ps = psq.tile([128, 512], F32, tag="ps_qkv")
                    for kt in range(KD):
                        nc.tensor.matmul(
                            ps[:],
                            w_kts[kt][:, s * 128 : s * 128 + 128],
                            x_nch[:, kt, :],
                            start=(kt == 0),
                            stop=(kt == KD - 1),
                        )
                    if s != 1:
                        # q/k section: rot = sec*cos + (Psw@sec)*sin
                        rot_idx = 4 if s == 0 else s - 2
                        sec_sb = secp.tile([HEAD_DIM, 512], F32R, tag="sec")
                        nc.vector.tensor_copy(sec_sb[:], ps[0:HEAD_DIM, :])
                        sw = pswp.tile([HEAD_DIM, 512], F32, tag="sw")
                        nc.tensor.matmul(
                            sw[:], psw_sb[:], sec_sb[:], start=True, stop=True
                        )
                        t_a = tmpp.tile([HEAD_DIM, 512], F32, tag="ta")
                        nc.gpsimd.tensor_mul(t_a[:], sec_sb[:], cos_sb[:, ncsl])
                        t_b = tmpp.tile([HEAD_DIM, 512], F32, tag="tb")
                        nc.vector.tensor_mul(t_b[:], sw[:], sin_sb[:, ncsl])
                        nc.vector.tensor_add(rot[:, rot_idx, ncsl], t_a[:], t_b[:])
                    else:
                        # v section: transpose to tokens-major, append ones col
                        v_sb = vsbp.tile([HEAD_DIM, 512], F32R, tag="v_sb")
                        nc.scalar.copy(v_sb[:], ps[0:HEAD_DIM, :])
                        for c in range(4):
                            kt_tok = nch * 4 + c
                            pst = ptr.tile([128, HEAD_DIM], F32R, tag="pst")
                            nc.tensor.transpose(
                                pst[:],
                                v_sb[:, c * 128 : (c + 1) * 128],
                                id_sb[0:HEAD_DIM, 0:HEAD_DIM],
                            )
                            nc.scalar.copy(v_aug[:, kt_tok, 0:HEAD_DIM], pst[:])

        # ---------------- phases B+C: attention + output proj, per q-chunk --
        with ExitStack() as sbc:
            cross_bc = sbc.enter_context(tc.tile_pool(name="cross_bc", bufs=1))
            wog_sb = cross_bc.tile([HEAD_DIM, NH, D_MODEL], F32R)
            nc.scalar.dma_start(
                out=wog_sb[:], in_=wogT[:].rearrange("(h f) e -> f h e", f=HEAD_DIM)
            )

            attnp = sbc.enter_context(tc.tile_pool(name="attnp", bufs=2))
            probsp = sbc.enter_context(tc.tile_pool(name="probsp", bufs=4))
            arawp = sbc.enter_context(tc.tile_pool(name="arawp", bufs=2))
            recipp = sbc.enter_context(tc.tile_pool(name="recipp", bufs=2))
            bcp = sbc.enter_context(tc.tile_pool(name="bcp", bufs=2))
            outp = sbc.enter_context(tc.tile_pool(name="outp", bufs=3))
            pscore = sbc.enter_context(
                tc.tile_pool(name="pscore", bufs=2, space="PSUM")
            )
            pattn = sbc.enter_context(tc.tile_pool(name="pattn", bufs=2, space="PSUM"))
            po = sbc.enter_context(tc.tile_pool(name="po", bufs=2, space="PSUM"))

            for qc in range(NCH):
                qsl = slice(qc * 512, (qc + 1) * 512)
                attnq = attnp.tile([HEAD_DIM, NH, 512], F32R, tag="attnq")
                for h in range(NH):
                    a_ps = pattn.tile([128, 512], F32, tag="a_ps")
                    for k2 in range(TT // 2):
                        s_ps = pscore.tile([128, 1024], F32, tag="s_ps")
                        for j in range(2):
                            kt = 2 * k2 + j
                            nc.tensor.matmul(
                                s_ps[:, j * 512 : (j + 1) * 512],
                                rot[:, 4, kt * 128 : (kt + 1) * 128],
                                rot[:, h, qsl],
                                start=True,
                                stop=True,
                            )
                        probs = probsp.tile([128, 1024], F32R, tag="probs")
                        nc.scalar.activation(probs[:], s_ps[:], act.Exp, scale=SCALE)
                        for j in range(2):
                            kt = 2 * k2 + j
                            nc.tensor.matmul(
                                a_ps[:],
                                v_aug[:, kt, :],
                                probs[:, j * 512 : (j + 1) * 512],
                                start=(kt == 0),
                                stop=(kt == TT - 1),
                            )
                    # normalize: attnq[h] = raw * broadcast(1/den)
                    den_sb = recipp.tile([1, 512], F32, tag="den")
                    nc.vector.tensor_copy(
                        den_sb[:], a_ps[HEAD_DIM : HEAD_DIM + 1, :]
                    )
                    recip = recipp.tile([1, 512], F32, tag="recip")
                    nc.vector.reciprocal_approx_fast(recip[:], den_sb[:])
                    bc_sb = bcp.tile([HEAD_DIM, 512], F32, tag="bc")
                    nc.gpsimd.partition_broadcast(bc_sb[:], recip[:])
                    araw = arawp.tile([HEAD_DIM, 512], F32, tag="araw")
                    nc.vector.tensor_copy(araw[:], a_ps[0:HEAD_DIM, :])
                    nc.vector.tensor_mul(attnq[:, h, :], araw[:], bc_sb[:])

                # output projection for this q-chunk's 512 tokens
                for tl in range(4):
                    o_sb = outp.tile([128, D_MODEL], F32, tag="o_sb")
                    for e in range(3):
                        o_ps = po.tile([128, 512], F32, tag="o_ps")
                        for h in range(NH):
                            nc.tensor.matmul(
                                o_ps[:],
                                attnq[:, h, tl * 128 : (tl + 1) * 128],
                                wog_sb[:, h, e * 512 : (e + 1) * 512],
                                start=(h == 0),
                                stop=(h == NH - 1),
                            )
                        nc.vector.tensor_copy(o_sb[:, e * 512 : (e + 1) * 512], o_ps[:])
                    row0 = qc * 512 + tl * 128
                    nc.sync.dma_start(
                        out=out_ext[row0 : row0 + 128, :], in_=o_sb[:]
                    )

    nc.finalize()
    return nc


def _rope_tables(grid_t, grid_h, grid_w):
    """cos/sin tables [96, 2048], dim-major, sign folded into sin."""
    t, h, w = np.meshgrid(
        np.arange(grid_t), np.arange(grid_h), np.arange(grid_w), indexing="ij"
    )
    pos = np.stack([t.reshape(-1), h.reshape(-1), w.reshape(-1)], axis=-1).astype(
        np.float64
    )  # [N, 3]
    dpa = HEAD_DIM // 3  # 32
    npairs = dpa // 2  # 16
    freqs = 1.0 / (THETA ** (np.arange(npairs, dtype=np.float64) * 2.0 / dpa))
    cos = np.zeros((HEAD_DIM, pos.shape[0]), dtype=np.float64)
    sin = np.zeros((HEAD_DIM, pos.shape[0]), dtype=np.float64)
    for axis in range(3):
        ang = pos[:, axis][None, :] * freqs[:, None]  # [npairs, N]
        c, s = np.cos(ang), np.sin(ang)
        base = axis * dpa
        cos[base + 0 : base + dpa : 2] = c
        cos[base + 1 : base + dpa : 2] = c
        sin[base + 0 : base + dpa : 2] = -s
        sin[base + 1 : base + dpa : 2] = s
    return cos.astype(np.float32), sin.astype(np.float32)


def _pair_swap():
    p = np.zeros((HEAD_DIM, HEAD_DIM), dtype=np.float32)
    for i in range(HEAD_DIM // 2):
        p[2 * i, 2 * i + 1] = 1.0
        p[2 * i + 1, 2 * i] = 1.0
    return p


def _run(x, w_qkv, w_o, grid_t, grid_h, grid_w, trace=False):
    x = np.asarray(x, dtype=np.float32)
    w_qkv = np.asarray(w_qkv, dtype=np.float32)
    w_o = np.asarray(w_o, dtype=np.float32)

    cos, sin = _rope_tables(int(grid_t), int(grid_h), int(grid_w))
    psw = _pair_swap()
    ident = np.eye(128, dtype=np.float32)

    q_dim = NUM_HEADS * HEAD_DIM  # 1536
    kv_dim = QUERY_GROUPS * HEAD_DIM  # 384

    in_maps = []
    for core in range(8):
        b, g = core // 4, core % 4
        # sections q0..q3 (head g*4+j), k(group g), v(group g), padded to 128 rows
        secs = [
            w_qkv[q_dim + g * HEAD_DIM : q_dim + (g + 1) * HEAD_DIM],
            w_qkv[q_dim + kv_dim + g * HEAD_DIM : q_dim + kv_dim + (g + 1) * HEAD_DIM],
        ]
        for j in range(NH):
            h = g * NH + j
            secs.append(w_qkv[h * HEAD_DIM : (h + 1) * HEAD_DIM])
        wsec = np.zeros((SEC * 128, D_MODEL), dtype=np.float32)
        for s, rows in enumerate(secs):
            wsec[s * 128 : s * 128 + HEAD_DIM] = rows
        import ml_dtypes

        bf16 = ml_dtypes.bfloat16
        in_maps.append(
            {
                "xT": np.ascontiguousarray(x[b].T).astype(bf16),
                "wqkvT": np.ascontiguousarray(wsec.T).astype(bf16),
                "wogT": np.ascontiguousarray(
                    w_o[:, g * kv_dim : (g + 1) * kv_dim].T
                ).astype(bf16),
                "cosT": cos.astype(bf16),
                "sinT": sin,
                "pswT": psw.astype(bf16),
                "ident": ident.astype(bf16),
            }
        )

    nc = _build_graph()
    res = run_bass_kernel_spmd(nc, in_maps, core_ids=list(range(8)), trace=trace)

    out = np.zeros((B, N, D_MODEL), dtype=np.float32)
    for core in range(8):
        out[core // 4] += res.results[core]["out"]
    return out, res


def kernel(x, w_qkv, w_o, grid_t, grid_h, grid_w):
    return _run(x, w_qkv, w_o, grid_t, grid_h, grid_w)[0]
